# revision 1
# baseline (speedup 1.0000x reference)
"""Causal self-attention (B=4, T=2048, C=1024, H=16) on 8 Trainium2 NeuronCores.

Sharding: tensor-parallel over heads. Each core owns 2 heads:
  - Wq/Wk/Wv column slices [C, 128], Wo row slice [128, C]
  - computes q/k/v for its heads from the full x, flash-style causal
    attention, and a partial output projection.
  - host sums the 8 partial outputs and adds bo.

Device algorithm per (batch b):
  xT[b] (pre-transposed on host to [C, T]) is DMA'd in 128-row tiles.
  qT/kT/vT [128(2 heads x 64d), T] = W.T @ xT  (+bias via ACT Identity)
  v_aug [128 t, 65] per head via PE transpose of vT (ones col appended).
  Scores (transposed): ST[j-tile 128, i-chunk 512] = kT.T @ qT per head,
    both heads packed with row-tiled matmuls (K=64 at rows 0 / 64).
  E = exp(ST) on ACT (1/sqrt(D) folded into Wq on host; no max-subtract
    needed: scores are O(5) so exp is safe in fp32).
  Causal mask on diagonal blocks via gpsimd affine_select (fill 0).
  yT_aug[65, i-chunk] += v_aug.T @ E accumulated over j in PSUM; row 64
    is the softmax denominator (ones column trick).
  alpha = 1/denominator (DVE reciprocal), broadcast across partitions
    (gpsimd partition_broadcast), yTa = yT * alpha (DVE, PSUM->SBUF).
  out_partial[i-tile 128, c-chunk 512] = sum_h yTa_h.T @ Wo_h, DMA'd out.

All matmuls run in float32r (full PE rate at free-dim >= 256).
"""

import sys

if "/opt/trn_rl_repo" not in sys.path:
    sys.path.insert(0, "/opt/trn_rl_repo")

from contextlib import ExitStack

import numpy as np

import concourse.bass as bass
import concourse.tile as tile
from concourse import bacc, mybir
from concourse import bass_utils
from concourse import masks

B, T, C, H, D = 4, 2048, 1024, 16, 64
N_CORES = 8
HPC = H // N_CORES  # heads per core = 2
W = HPC * D  # per-core projection width = 128

F32 = mybir.dt.float32
F32R = mybir.dt.float32r
F16 = mybir.dt.float16
AF = mybir.ActivationFunctionType

ICH = 512  # i (query) chunk in the free dim
LOG16 = float(np.log(16.0))
NIC = T // ICH  # 4
NKT = C // 128  # 8 contraction tiles for projections
NJT = T // 128  # 16 key tiles

_CACHE = {}


def _kernel_body(ctx, tc, xT, wq, wk, wv, wo0, wo1, bq, bk, bv, ones, ones16r, out):
    nc = tc.nc

    const_p = ctx.enter_context(tc.tile_pool(name="const", bufs=1))
    w_p = ctx.enter_context(tc.tile_pool(name="wts", bufs=1))
    xt_p = ctx.enter_context(tc.tile_pool(name="xt", bufs=NKT))
    act_p = ctx.enter_context(tc.tile_pool(name="acts", bufs=2))
    va_p = ctx.enter_context(tc.tile_pool(name="vaug", bufs=20))
    e_p = ctx.enter_context(tc.tile_pool(name="ep", bufs=5))
    yta_p = ctx.enter_context(tc.tile_pool(name="yta", bufs=8))
    r_p = ctx.enter_context(tc.tile_pool(name="alpha", bufs=4))
    ob_p = ctx.enter_context(tc.tile_pool(name="ob", bufs=4))
    pmisc = ctx.enter_context(tc.tile_pool(name="pmisc", bufs=3, space="PSUM"))
    ps_p = ctx.enter_context(tc.tile_pool(name="psc", bufs=3, space="PSUM"))
    py_p = ctx.enter_context(tc.tile_pool(name="py", bufs=2, space="PSUM"))

    # constants / weights (loaded once)
    ident = const_p.tile([128, 128], F16, tag="ident")
    masks.make_identity(nc, ident[:])

    bias_q = const_p.tile([W, 1], F32, tag="bq")
    bias_k = const_p.tile([W, 1], F32, tag="bk")
    bias_v = const_p.tile([W, 1], F32, tag="bv")
    ones_sb = const_p.tile([128, 64], F32, tag="ones")
    nc.sync.dma_start(ones_sb[:], ones[:])
    ones16 = const_p.tile([128, 64], F16, tag="ones16")
    nc.sync.dma_start(ones16[:], ones16r[:])
    log16 = const_p.tile([128, 1], F32, tag="log16")
    nc.gpsimd.memset(log16[:], LOG16)
    nc.sync.dma_start(bias_q[:], bq[:])
    nc.sync.dma_start(bias_k[:], bk[:])
    nc.sync.dma_start(bias_v[:], bv[:])

    # projection weights: [128 c-part, 128 d] per k-tile, packed along free dim
    wq_sb = w_p.tile([128, C], F16, tag="wq")
    wk_sb = w_p.tile([128, C], F16, tag="wk")
    wv_sb = w_p.tile([128, C], F16, tag="wv")
    for kt in range(NKT):
        sl = slice(kt * 128, (kt + 1) * 128)
        nc.sync.dma_start(wq_sb[:, sl], wq[sl, :])
        nc.sync.dma_start(wk_sb[:, sl], wk[sl, :])
        nc.sync.dma_start(wv_sb[:, sl], wv[sl, :])
    wo0_sb = w_p.tile([D, C], F16, tag="wo0")
    wo1_sb = w_p.tile([D, C], F16, tag="wo1")
    nc.sync.dma_start(wo0_sb[:], wo0[:])
    nc.sync.dma_start(wo1_sb[:], wo1[:])

    pending_outproj = []
    for b in range(B):
        # ---- load xT[b] ----
        xts = []
        for kt in range(NKT):
            xt = xt_p.tile([128, T], F16, tag="xt")
            nc.sync.dma_start(xt[:], xT[b, kt * 128 : (kt + 1) * 128, :])
            xts.append(xt)

        # ---- projections: qT/kT/vT [128, T] ----
        qT = act_p.tile([128, T], F16, tag="qT")
        kT = act_p.tile([128, T], F16, tag="kT")
        vT = act_p.tile([128, T], F16, tag="vT")
        for n in range(NIC):
            csl = slice(n * ICH, (n + 1) * ICH)
            psq = pmisc.tile([128, ICH], F32, tag="pp")
            psk = pmisc.tile([128, ICH], F32, tag="pp")
            psv = pmisc.tile([128, ICH], F32, tag="pp")
            for kt in range(NKT):
                wsl = slice(kt * 128, (kt + 1) * 128)
                st, sp = kt == 0, kt == NKT - 1
                nc.tensor.matmul(psq[:], wq_sb[:, wsl], xts[kt][:, csl], start=st, stop=sp)
                nc.tensor.matmul(psk[:], wk_sb[:, wsl], xts[kt][:, csl], start=st, stop=sp)
                nc.tensor.matmul(psv[:], wv_sb[:, wsl], xts[kt][:, csl], start=st, stop=sp)
            nc.vector.tensor_scalar_add(qT[:, csl], psq[:], bias_q[:])
            nc.vector.tensor_scalar_add(kT[:, csl], psk[:], bias_k[:])
            nc.vector.tensor_scalar_add(vT[:, csl], psv[:], bias_v[:])

        # ---- v_aug tiles: [128 t, 130] = [h0 d64 | ones | h1 d64 | ones] ----
        vas = []
        for tt in range(NJT):
            if tt == 4 and pending_outproj:
                pending_outproj.pop(0)()
            pst = ps_p.tile([128, 128], F16, tag="ps")
            nc.tensor.transpose(pst[:], vT[:, tt * 128 : (tt + 1) * 128], ident[:])
            va = va_p.tile([128, 130], F16, tag="va")
            nc.vector.tensor_copy(va[:, 0:64], pst[:, 0:64])
            nc.vector.tensor_copy(va[:, 65:129], pst[:, 64:128])
            nc.vector.tensor_copy(va[:, 64:65], ones16[:, 0:1])
            nc.vector.tensor_copy(va[:, 129:130], ones16[:, 0:1])
            vas.append(va)

        # ---- attention + normalization per i-chunk ----
        ytas = []  # [(yta_h0, yta_h1)] per ic, each [64, ICH] fp16
        for ic in range(NIC):
            i0 = ic * ICH
            isl = slice(i0, i0 + ICH)
            njt = (i0 + ICH) // 128
            py0 = py_p.tile([65, ICH], F32, tag="py")
            py1 = py_p.tile([65, ICH], F32, tag="py")
            for jt in range(njt):
                if jt == 2 and pending_outproj:
                    pending_outproj.pop(0)()
                j0 = jt * 128
                jsl = slice(j0, j0 + 128)
                ps0 = ps_p.tile([128, ICH], F32, tag="ps")
                ps1 = ps_p.tile([128, ICH], F32, tag="ps")
                nc.tensor.matmul(
                    ps0[:], kT[0:64, jsl], qT[0:64, isl],
                    start=True, stop=True, tile_position=(0, 0),
                )
                nc.tensor.matmul(
                    ps1[:], kT[64:128, jsl], qT[64:128, isl],
                    start=True, stop=True, tile_position=(64, 0),
                )
                e0 = e_p.tile([128, ICH], F16, tag="e")
                e1 = e_p.tile([128, ICH], F16, tag="e")
                nc.scalar.activation(e0[:], ps0[:], AF.Exp)
                nc.scalar.activation(e1[:], ps1[:], AF.Exp)
                if j0 + 127 > i0:  # diagonal block: zero where j > i
                    for e in (e0, e1):
                        nc.gpsimd.affine_select(
                            out=e[:], in_=e[:],
                            pattern=[[1, ICH]],
                            compare_op=mybir.AluOpType.is_ge,
                            fill=0.0,
                            base=i0 - j0,
                            channel_multiplier=-1,
                        )
                st, sp = jt == 0, jt == njt - 1
                nc.tensor.matmul(py0[:], vas[jt][:, 0:65], e0[:], start=st, stop=sp)
                nc.tensor.matmul(py1[:], vas[jt][:, 65:130], e1[:], start=st, stop=sp)

            pair = []
            lnts = []
            for py in (py0, py1):
                lnt = r_p.tile([65, ICH], F32, tag="lnt")
                nc.scalar.activation(lnt[64:65, :], py[64:65, :], AF.Ln)
                lnts.append(lnt)
            for py, lnt in zip((py0, py1), lnts):
                # r = 16/denom (x16 keeps r in fp16-normal range; /16 folded into Wo)
                r = r_p.tile([65, ICH], F16, tag="r")
                nc.scalar.activation(r[64:65, :], lnt[64:65, :], AF.Exp, scale=-1.0, bias=log16[64:65, :])
                ab = ps_p.tile([64, ICH], F32, tag="ps")
                nc.tensor.matmul(
                    ab[:], ones16[64:65, :], r[64:65, :],
                    start=True, stop=True, tile_position=(64, 0),
                )
                ab_sb = r_p.tile([64, ICH], F32, tag="absb")
                nc.vector.tensor_copy(ab_sb[:], ab[:])
                yta = yta_p.tile([64, ICH], F16, tag="yta")
                nc.vector.tensor_mul(yta[:], py[0:64, :], ab_sb[:])
                pair.append(yta)
            ytas.append(pair)

            # out-projection for this i-chunk: queued, emitted one phase later
            # so the in-order PE stream has next-chunk scores to run while the
            # softmax chain (Ln/Exp/bcast/mult) completes.
            def _outproj(b=b, ic=ic, y0=pair[0], y1=pair[1]):
                for itl in range(ICH // 128):
                    off = itl * 128
                    it = ic * 4 + itl
                    for nch in range(C // ICH):
                        osl = slice(nch * ICH, (nch + 1) * ICH)
                        po = pmisc.tile([128, ICH], F32, tag="pp")
                        nc.tensor.matmul(
                            po[:], y0[:, off : off + 128], wo0_sb[:, osl], start=True, stop=False
                        )
                        nc.tensor.matmul(
                            po[:], y1[:, off : off + 128], wo1_sb[:, osl], start=False, stop=True
                        )
                        ob = ob_p.tile([128, ICH], F32, tag="ob")
                        nc.vector.tensor_copy(ob[:], po[:])
                        nc.sync.dma_start(out[b, it * 128 : (it + 1) * 128, osl], ob[:])
            pending_outproj.append(_outproj)

    # flush remaining queued out-projections
    while pending_outproj:
        pending_outproj.pop(0)()


def _build():
    if "nc" in _CACHE:
        return _CACHE["nc"]
    nc = bacc.Bacc("TRN2", target_bir_lowering=False, debug=False, num_devices=N_CORES)
    xT = nc.dram_tensor("xT", [B, C, T], F16, kind="ExternalInput").ap()
    wq = nc.dram_tensor("wq", [C, W], F16, kind="ExternalInput").ap()
    wk = nc.dram_tensor("wk", [C, W], F16, kind="ExternalInput").ap()
    wv = nc.dram_tensor("wv", [C, W], F16, kind="ExternalInput").ap()
    wo0 = nc.dram_tensor("wo0", [D, C], F16, kind="ExternalInput").ap()
    wo1 = nc.dram_tensor("wo1", [D, C], F16, kind="ExternalInput").ap()
    bq = nc.dram_tensor("bq", [W, 1], F32, kind="ExternalInput").ap()
    bk = nc.dram_tensor("bk", [W, 1], F32, kind="ExternalInput").ap()
    bv = nc.dram_tensor("bv", [W, 1], F32, kind="ExternalInput").ap()
    ones = nc.dram_tensor("ones", [128, 64], F32, kind="ExternalInput").ap()
    ones16r = nc.dram_tensor("ones16r", [128, 64], F16, kind="ExternalInput").ap()
    out = nc.dram_tensor("out", [B, T, C], F32, kind="ExternalOutput").ap()

    with tile.TileContext(nc) as tc:
        with ExitStack() as ctx:
            _kernel_body(ctx, tc, xT, wq, wk, wv, wo0, wo1, bq, bk, bv, ones, ones16r, out)
    nc.compile()
    _CACHE["nc"] = nc
    return nc


def make_in_maps(inputs):
    x = np.asarray(inputs["x"], np.float32)
    Wq = np.asarray(inputs["Wq"], np.float32)
    bq = np.asarray(inputs["bq"], np.float32)
    Wk = np.asarray(inputs["Wk"], np.float32)
    bk = np.asarray(inputs["bk"], np.float32)
    Wv = np.asarray(inputs["Wv"], np.float32)
    bv = np.asarray(inputs["bv"], np.float32)
    Wo = np.asarray(inputs["Wo"], np.float32)

    scale = np.float32(1.0 / np.sqrt(D))
    xT = np.ascontiguousarray(x.transpose(0, 2, 1))  # [B, C, T]
    Wq_s = Wq * scale
    bq_s = bq * scale

    in_maps = []
    for c in range(N_CORES):
        s = slice(c * W, (c + 1) * W)
        in_maps.append(
            {
                "xT": xT.astype(np.float16),
                "wq": np.ascontiguousarray(Wq_s[:, s]).astype(np.float16),
                "wk": np.ascontiguousarray(Wk[:, s]).astype(np.float16),
                "wv": np.ascontiguousarray(Wv[:, s]).astype(np.float16),
                "wo0": np.ascontiguousarray(Wo[c * W : c * W + D, :] / 16.0).astype(np.float16),
                "wo1": np.ascontiguousarray(Wo[c * W + D : (c + 1) * W, :] / 16.0).astype(np.float16),
                "bq": np.ascontiguousarray(bq_s[s, None]),
                "bk": np.ascontiguousarray(bk[s, None]),
                "bv": np.ascontiguousarray(bv[s, None]),
                "ones": np.ones((128, 64), np.float32),
                "ones16r": np.ones((128, 64), np.float16),
            }
        )
    return in_maps


def kernel(**inputs):
    nc = _build()
    in_maps = make_in_maps(inputs)
    res = bass_utils.run_bass_kernel_spmd(nc, in_maps, core_ids=list(range(N_CORES)))
    bo = np.asarray(inputs["bo"], np.float32)
    out = np.zeros((B, T, C), np.float32)
    for c in range(N_CORES):
        out += res.results[c]["out"]
    out += bo
    return out


if __name__ == "__main__":
    rng = np.random.default_rng(0)
    ins = {
        "x": rng.standard_normal((B, T, C), dtype=np.float32),
        "Wq": rng.standard_normal((C, C), dtype=np.float32) / 32,
        "bq": rng.standard_normal((C,), dtype=np.float32) * 0.02,
        "Wk": rng.standard_normal((C, C), dtype=np.float32) / 32,
        "bk": rng.standard_normal((C,), dtype=np.float32) * 0.02,
        "Wv": rng.standard_normal((C, C), dtype=np.float32) / 32,
        "bv": rng.standard_normal((C,), dtype=np.float32) * 0.02,
        "Wo": rng.standard_normal((C, C), dtype=np.float32) / 32,
        "bo": rng.standard_normal((C,), dtype=np.float32) * 0.02,
    }
    got = kernel(**ins)
    print("kernel ran, out shape", got.shape)



# revision 4
# speedup vs baseline: 1.7414x; 1.7414x over previous
"""Causal self-attention (B=4, T=2048, C=1024, H=16) on 8 Trainium2 NeuronCores.

Sharding: tensor-parallel over heads. Each core owns 2 heads:
  - Wq/Wk column slices [C, 128] (Wq pre-scaled by 1/sqrt(D)), Wv augmented
    to [C, 130] with two zero columns whose biases are 1.0 (so the "ones"
    denominator column of v_aug comes straight out of the projection),
    Wo row slice [128, C].
  - computes q/k/v for its heads from the full x, flash-style causal
    attention, and a partial output projection (f16); host sums partials.

Per-core schedule (single launch, software-pipelined across batches):
  qT/kT [128(2h x 64d), T] = W.T @ xT   (+bias per-partition via DVE)
  v_aug [128 t, 130]       = xT_tile.T @ Wv_aug + bias_row (DVE add)
  Scores (transposed): psc[j 128, i-chunk<=512] = kT.T @ qT per head,
    heads row-packed at tile_position (0,0)/(64,0) into the two banks of a
    [128, 1024] PSUM tile. Diagonal j-tiles narrowed to the valid i-range
    (widths 512/384/256/128) - 15% less score/exp/PV work.
  e = exp(s - 4) on ACT, one ACTIVATE per step spanning both banks
    (the -4 shift keeps 1/denom in fp16-friendly range; cancels in softmax).
  Causal triangle on the 128-col diagonal edge via gpsimd affine_select.
  py[65, i] += v_aug.T @ e per head (PSUM accumulate over j-tiles); row 64
    is the softmax denominator.
  alpha = reciprocal_approx_fast(denom) (DVE), partition_broadcast (gpsimd),
    yta[128 d2h, i] = py * alpha (DVE, writes both heads into one tile -
    partition-offset DVE write verified on HW).
  out[i 128, c-chunk 512] = yta.T @ Wo, one K=128 matmul, f16 out via DVE.

PE density (HAM warmth): projections of batch b+1 and out-projections of
batch b are emitted as filler work interleaved into attention(b)'s
ACT-bound inner loop; scores are emitted one step ahead of PV so the PE
never sits behind an exp dependency.
"""

import sys

if "/opt/trn_rl_repo" not in sys.path:
    sys.path.insert(0, "/opt/trn_rl_repo")

from collections import deque
from contextlib import ExitStack

import numpy as np

import concourse.bass as bass
import concourse.tile as tile
from concourse import bacc, mybir
from concourse import bass_utils

B, T, C, H, D = 4, 2048, 1024, 16, 64
N_CORES = 8
HPC = H // N_CORES  # heads per core = 2
W = HPC * D  # per-core projection width = 128

F32 = mybir.dt.float32
F16 = mybir.dt.float16
AF = mybir.ActivationFunctionType

ICH = 512  # i (query) chunk in the free dim
NIC = T // ICH  # 4
NKT = C // 128  # 8 contraction tiles for projections
NTT = T // 128  # 16 t-tiles (keys) per batch
M0 = 4.0  # constant score shift inside exp; cancels in softmax

_CACHE = {}


def _kernel_body(ctx, tc, xT, wq, wk, wva, wo, bq, bk, bva, out):
    nc = tc.nc

    const_p = ctx.enter_context(tc.tile_pool(name="const", bufs=1))
    w_p = ctx.enter_context(tc.tile_pool(name="wts", bufs=1))
    xt_p = ctx.enter_context(tc.tile_pool(name="xt", bufs=2 * NKT))
    qk_p = ctx.enter_context(tc.tile_pool(name="qk", bufs=2))
    va_p = ctx.enter_context(tc.tile_pool(name="vaug", bufs=2 * NTT))
    e_p = ctx.enter_context(tc.tile_pool(name="ep", bufs=3))
    yta_p = ctx.enter_context(tc.tile_pool(name="yta", bufs=3))
    r_p = ctx.enter_context(tc.tile_pool(name="rp", bufs=4))
    rb_p = ctx.enter_context(tc.tile_pool(name="rbp", bufs=4))
    ob_p = ctx.enter_context(tc.tile_pool(name="ob", bufs=6))
    psc_p = ctx.enter_context(tc.tile_pool(name="psc", bufs=2, space="PSUM"))
    py_p = ctx.enter_context(tc.tile_pool(name="py", bufs=2, space="PSUM"))
    pm_p = ctx.enter_context(tc.tile_pool(name="pm", bufs=2, space="PSUM"))

    # ---- constants / weights (loaded once) ----
    bias_q = const_p.tile([W, 1], F32, tag="bq")
    bias_k = const_p.tile([W, 1], F32, tag="bk")
    nc.sync.dma_start(bias_q[:], bq[:])
    nc.sync.dma_start(bias_k[:], bk[:])
    bva_row = const_p.tile([1, 130], F32, tag="bvar")
    nc.sync.dma_start(bva_row[:], bva[:])
    bva_bc = const_p.tile([128, 130], F32, tag="bvab")
    nc.gpsimd.partition_broadcast(bva_bc[:], bva_row[:])
    m0t = const_p.tile([128, 1], F32, tag="m0")
    nc.gpsimd.memset(m0t[:], -M0)

    wq_sb = w_p.tile([128, C], F16, tag="wq")
    wk_sb = w_p.tile([128, C], F16, tag="wk")
    wva_sb = w_p.tile([128, NKT * 130], F16, tag="wva")
    for kt in range(NKT):
        sl = slice(kt * 128, (kt + 1) * 128)
        nc.sync.dma_start(wq_sb[:, sl], wq[sl, :])
        nc.sync.dma_start(wk_sb[:, sl], wk[sl, :])
        nc.sync.dma_start(wva_sb[:, kt * 130 : (kt + 1) * 130], wva[sl, :])
    wo_sb = w_p.tile([128, C], F16, tag="wo")
    nc.sync.dma_start(wo_sb[:], wo[:])

    xts = [[None] * NKT for _ in range(B)]
    qTs = [None] * B
    kTs = [None] * B
    vas = [[None] * NTT for _ in range(B)]

    qproj = deque()
    qout = deque()
    state = {"step": 0}
    NSTEPS = sum(4 * ic + 4 for ic in range(NIC))  # 40

    def emit_load(b):
        for kt in range(NKT):
            xt = xt_p.tile([128, T], F16, tag="xt")
            nc.sync.dma_start(xt[:], xT[b, kt * 128 : (kt + 1) * 128, :])
            xts[b][kt] = xt

    def make_proj_thunks(b):
        ths = []

        def alloc(b=b):
            qTs[b] = qk_p.tile([128, T], F16, tag="qT", name="qT")
            kTs[b] = qk_p.tile([128, T], F16, tag="kT", name="kT")

        ths.append(alloc)
        for n in range(NIC):
            for which in ("q", "k"):
                def th(b=b, n=n, which=which):
                    csl = slice(n * ICH, (n + 1) * ICH)
                    wsb = wq_sb if which == "q" else wk_sb
                    bias = bias_q if which == "q" else bias_k
                    dst = qTs[b] if which == "q" else kTs[b]
                    ps = pm_p.tile([128, ICH], F32, tag="pm")
                    for kt in range(NKT):
                        nc.tensor.matmul(
                            ps[:],
                            wsb[:, kt * 128 : (kt + 1) * 128],
                            xts[b][kt][:, csl],
                            start=kt == 0,
                            stop=kt == NKT - 1,
                        )
                    nc.vector.tensor_scalar_add(dst[:, csl], ps[:], bias[:])

                ths.append(th)
        for tt in range(NTT):
            def th(b=b, tt=tt):
                tsl = slice(tt * 128, (tt + 1) * 128)
                ps = pm_p.tile([128, 130], F32, tag="pm")
                for kt in range(NKT):
                    nc.tensor.matmul(
                        ps[:],
                        xts[b][kt][:, tsl],
                        wva_sb[:, kt * 130 : (kt + 1) * 130],
                        start=kt == 0,
                        stop=kt == NKT - 1,
                    )
                va = va_p.tile([128, 130], F16, tag="va")
                nc.vector.tensor_add(va[:], ps[:], bva_bc[:])
                vas[b][tt] = va

            ths.append(th)
        return ths

    def make_outproj_thunks(b, ic, yta):
        ths = []
        for itl in range(4):
            def th(b=b, ic=ic, yta=yta, itl=itl):
                it = ic * 4 + itl
                off = itl * 128
                for nch in range(2):
                    osl = slice(nch * ICH, (nch + 1) * ICH)
                    po = pm_p.tile([128, ICH], F32, tag="pm")
                    nc.tensor.matmul(
                        po[:], yta[:, off : off + 128], wo_sb[:, osl],
                        start=True, stop=True,
                    )
                    obt = ob_p.tile([128, ICH], F16, tag="ob")
                    nc.vector.tensor_copy(obt[:], po[:])
                    nc.sync.dma_start(out[b, it * 128 : (it + 1) * 128, osl], obt[:])

            ths.append(th)
        return ths

    def pace():
        s = state["step"]
        state["step"] = s + 1
        if s >= 8 and qproj:
            if s < NSTEPS - 4:
                k = -(-len(qproj) // (NSTEPS - 4 - s))
            else:
                k = len(qproj)
            for _ in range(min(k, len(qproj))):
                qproj.popleft()()
        if qout:
            qout.popleft()()

    def emit_alpha_yta(b, ic, py0, py1):
        yta = yta_p.tile([128, ICH], F16, tag="yta")
        for h, py in ((0, py0), (1, py1)):
            dn = r_p.tile([1, ICH], F32, tag="dn")
            # custom-DVE reciprocal misreads PSUM sources; stage via SBUF
            nc.vector.tensor_copy(dn[:], py[64:65, :])
            r = r_p.tile([1, ICH], F32, tag="r")
            nc.vector.reciprocal_approx_fast(r[:], dn[:])
            rb = rb_p.tile([64, ICH], F32, tag="rb")
            nc.gpsimd.partition_broadcast(rb[:], r[:])
            nc.vector.tensor_mul(yta[h * 64 : (h + 1) * 64, :], py[0:64, :], rb[:])
        qout.extend(make_outproj_thunks(b, ic, yta))

    def do_batch_attention(b):
        steps = []
        for ic in range(NIC):
            njt = 4 * ic + 4
            i0 = ic * ICH
            for jt in range(njt):
                k = jt - 4 * ic
                if k >= 0:
                    wdt, istart = ICH - 128 * k, 128 * jt
                else:
                    wdt, istart = ICH, i0
                steps.append((ic, jt, njt, i0, wdt, istart))

        pscs = {}

        def emit_scores(si):
            ic, jt, njt, i0, wdt, istart = steps[si]
            psc = psc_p.tile([128, 1024], F32, tag="psc")
            jsl = slice(jt * 128, jt * 128 + 128)
            isl = slice(istart, i0 + ICH)
            nc.tensor.matmul(
                psc[:, 0:wdt], kTs[b][0:64, jsl], qTs[b][0:64, isl],
                start=True, stop=True, tile_position=(0, 0),
            )
            nc.tensor.matmul(
                psc[:, 512 : 512 + wdt], kTs[b][64:128, jsl], qTs[b][64:128, isl],
                start=True, stop=True, tile_position=(64, 0),
            )
            pscs[si] = psc

        pys = {}
        emit_scores(0)
        for si in range(len(steps)):
            ic, jt, njt, i0, wdt, istart = steps[si]
            if si + 1 < len(steps):
                emit_scores(si + 1)
            psc = pscs.pop(si)
            e = e_p.tile([128, 1024], F16, tag="e")
            nc.scalar.activation(e[:, 0 : 512 + wdt], psc[:, 0 : 512 + wdt], AF.Exp, bias=m0t[:])
            if jt - 4 * ic >= 0:  # diagonal: zero the j > i triangle (128 cols)
                for off in (0, 512):
                    nc.gpsimd.affine_select(
                        out=e[:, off : off + 128],
                        in_=e[:, off : off + 128],
                        pattern=[[1, 128]],
                        compare_op=mybir.AluOpType.is_ge,
                        fill=0.0,
                        base=0,
                        channel_multiplier=-1,
                    )
            if jt == 0:
                pys[ic] = (
                    py_p.tile([65, ICH], F32, tag="py", name="py0"),
                    py_p.tile([65, ICH], F32, tag="py", name="py1"),
                )
            py0, py1 = pys[ic]
            coff = istart - i0
            st, sp = jt == 0, jt == njt - 1
            nc.tensor.matmul(
                py0[:, coff:ICH], vas[b][jt][:, 0:65], e[:, 0:wdt], start=st, stop=sp
            )
            nc.tensor.matmul(
                py1[:, coff:ICH], vas[b][jt][:, 65:130], e[:, 512 : 512 + wdt],
                start=st, stop=sp,
            )
            if sp:
                emit_alpha_yta(b, ic, *pys.pop(ic))
            pace()

    # ---- pipeline over batches ----
    emit_load(0)
    for th in make_proj_thunks(0):
        th()
    for b in range(B):
        if b + 1 < B:
            emit_load(b + 1)
            qproj.extend(make_proj_thunks(b + 1))
        state["step"] = 0
        do_batch_attention(b)
        while qproj:  # proj(b+1) must be complete before attention(b+1)
            qproj.popleft()()
    while qout:
        qout.popleft()()


def _build():
    if "nc" in _CACHE:
        return _CACHE["nc"]
    nc = bacc.Bacc("TRN2", target_bir_lowering=False, debug=False, num_devices=N_CORES)
    xT = nc.dram_tensor("xT", [B, C, T], F16, kind="ExternalInput").ap()
    wq = nc.dram_tensor("wq", [C, W], F16, kind="ExternalInput").ap()
    wk = nc.dram_tensor("wk", [C, W], F16, kind="ExternalInput").ap()
    wva = nc.dram_tensor("wva", [C, 130], F16, kind="ExternalInput").ap()
    wo = nc.dram_tensor("wo", [W, C], F16, kind="ExternalInput").ap()
    bq = nc.dram_tensor("bq", [W, 1], F32, kind="ExternalInput").ap()
    bk = nc.dram_tensor("bk", [W, 1], F32, kind="ExternalInput").ap()
    bva = nc.dram_tensor("bva", [1, 130], F32, kind="ExternalInput").ap()
    out = nc.dram_tensor("out", [B, T, C], F16, kind="ExternalOutput").ap()

    with tile.TileContext(nc) as tc:
        with ExitStack() as ctx:
            _kernel_body(ctx, tc, xT, wq, wk, wva, wo, bq, bk, bva, out)
    nc.compile()
    _CACHE["nc"] = nc
    return nc


def make_in_maps(inputs):
    x = np.asarray(inputs["x"], np.float32)
    Wq = np.asarray(inputs["Wq"], np.float32)
    bq = np.asarray(inputs["bq"], np.float32)
    Wk = np.asarray(inputs["Wk"], np.float32)
    bk = np.asarray(inputs["bk"], np.float32)
    Wv = np.asarray(inputs["Wv"], np.float32)
    bv = np.asarray(inputs["bv"], np.float32)
    Wo = np.asarray(inputs["Wo"], np.float32)

    scale = np.float32(1.0 / np.sqrt(D))
    xT = np.ascontiguousarray(x.transpose(0, 2, 1)).astype(np.float16)  # [B, C, T]
    Wq_s = Wq * scale
    bq_s = bq * scale

    in_maps = []
    for c in range(N_CORES):
        s = slice(c * W, (c + 1) * W)
        wva = np.zeros((C, 130), np.float32)
        wva[:, 0:64] = Wv[:, c * W : c * W + D]
        wva[:, 65:129] = Wv[:, c * W + D : (c + 1) * W]
        bva = np.zeros((1, 130), np.float32)
        bva[0, 0:64] = bv[c * W : c * W + D]
        bva[0, 64] = 1.0
        bva[0, 65:129] = bv[c * W + D : (c + 1) * W]
        bva[0, 129] = 1.0
        in_maps.append(
            {
                "xT": xT,
                "wq": np.ascontiguousarray(Wq_s[:, s]).astype(np.float16),
                "wk": np.ascontiguousarray(Wk[:, s]).astype(np.float16),
                "wva": wva.astype(np.float16),
                "wo": np.ascontiguousarray(Wo[s, :]).astype(np.float16),
                "bq": np.ascontiguousarray(bq_s[s, None]),
                "bk": np.ascontiguousarray(bk[s, None]),
                "bva": bva,
            }
        )
    return in_maps


def kernel(**inputs):
    nc = _build()
    in_maps = make_in_maps(inputs)
    res = bass_utils.run_bass_kernel_spmd(nc, in_maps, core_ids=list(range(N_CORES)))
    bo = np.asarray(inputs["bo"], np.float32)
    out = np.zeros((B, T, C), np.float32)
    for c in range(N_CORES):
        out += res.results[c]["out"].astype(np.float32)
    out += bo
    return out


if __name__ == "__main__":
    rng = np.random.default_rng(0)
    ins = {
        "x": rng.standard_normal((B, T, C), dtype=np.float32),
        "Wq": rng.standard_normal((C, C), dtype=np.float32) / 32,
        "bq": rng.standard_normal((C,), dtype=np.float32) * 0.02,
        "Wk": rng.standard_normal((C, C), dtype=np.float32) / 32,
        "bk": rng.standard_normal((C,), dtype=np.float32) * 0.02,
        "Wv": rng.standard_normal((C, C), dtype=np.float32) / 32,
        "bv": rng.standard_normal((C,), dtype=np.float32) * 0.02,
        "Wo": rng.standard_normal((C, C), dtype=np.float32) / 32,
        "bo": rng.standard_normal((C,), dtype=np.float32) * 0.02,
    }
    got = kernel(**ins)
    print("kernel ran, out shape", got.shape)


# revision 10
# speedup vs baseline: 1.8558x; 1.0657x over previous
"""Causal self-attention (B=4, T=2048, C=1024, H=16) on 8 Trainium2 NeuronCores.

Sharding: tensor-parallel over heads. Each core owns 2 heads:
  - Wq/Wk column slices [C, 128] (Wq pre-scaled by 1/sqrt(D)), Wv augmented
    to [C, 130] with two zero columns whose biases are 1.0 (so the "ones"
    denominator column of v_aug comes straight out of the projection),
    Wo row slice [128, C].
  - computes q/k/v for its heads from the full x, flash-style causal
    attention, and a partial output projection (f16); host sums partials.

Per-core schedule (single launch, software-pipelined across batches):
  qT/kT [128(2h x 64d), T] = W.T @ xT   (+bias per-partition via DVE)
  v_aug [128 t, 130]       = xT_tile.T @ Wv_aug + bias_row (DVE add)
  Scores (transposed): psc[j 128, i-chunk<=512] = kT.T @ qT per head,
    heads row-packed at tile_position (0,0)/(64,0) into the two banks of a
    [128, 1024] PSUM tile. Diagonal j-tiles narrowed to the valid i-range
    (widths 512/384/256/128) - 15% less score/exp/PV work.
  e = exp(s - 4) on ACT, one ACTIVATE per step spanning both banks
    (the -4 shift keeps 1/denom in fp16-friendly range; cancels in softmax).
  Causal triangle on the 128-col diagonal edge via gpsimd affine_select.
  py[65, i] += v_aug.T @ e per head (PSUM accumulate over j-tiles); row 64
    is the softmax denominator.
  alpha = reciprocal_approx_fast(denom) (DVE), partition_broadcast (gpsimd),
    yta[128 d2h, i] = py * alpha (DVE, writes both heads into one tile -
    partition-offset DVE write verified on HW).
  out[i 128, c-chunk 512] = yta.T @ Wo, one K=128 matmul, f16 out via DVE.

PE density (HAM warmth): projections of batch b+1 and out-projections of
batch b are emitted as filler work interleaved into attention(b)'s
ACT-bound inner loop; scores are emitted one step ahead of PV so the PE
never sits behind an exp dependency.
"""

import sys

if "/opt/trn_rl_repo" not in sys.path:
    sys.path.insert(0, "/opt/trn_rl_repo")

from collections import deque
from contextlib import ExitStack

import numpy as np

import concourse.bass as bass
import concourse.tile as tile
from concourse import bacc, mybir
from concourse import bass_utils

B, T, C, H, D = 4, 2048, 1024, 16, 64
N_CORES = 8
HPC = H // N_CORES  # heads per core = 2
W = HPC * D  # per-core projection width = 128

F32 = mybir.dt.float32
F16 = mybir.dt.float16
AF = mybir.ActivationFunctionType

ICH = 512  # i (query) chunk in the free dim
NIC = T // ICH  # 4
NKT = C // 128  # 8 contraction tiles for projections
NTT = T // 128  # 16 t-tiles (keys) per batch
M0 = 4.0  # constant score shift inside exp; cancels in softmax

_CACHE = {}


def _kernel_body(ctx, tc, xT, wq, wk, wva, wo, bq, bk, bva, out):
    nc = tc.nc

    const_p = ctx.enter_context(tc.tile_pool(name="const", bufs=1))
    w_p = ctx.enter_context(tc.tile_pool(name="wts", bufs=1))
    xt_p = ctx.enter_context(tc.tile_pool(name="xt", bufs=2 * NKT))
    qk_p = ctx.enter_context(tc.tile_pool(name="qk", bufs=2))
    va_p = ctx.enter_context(tc.tile_pool(name="vaug", bufs=2 * NTT))
    e_p = ctx.enter_context(tc.tile_pool(name="ep", bufs=3))
    yta_p = ctx.enter_context(tc.tile_pool(name="yta", bufs=3))
    r_p = ctx.enter_context(tc.tile_pool(name="rp", bufs=4))
    rb_p = ctx.enter_context(tc.tile_pool(name="rbp", bufs=4))
    ob_p = ctx.enter_context(tc.tile_pool(name="ob", bufs=6))
    psc_p = ctx.enter_context(tc.tile_pool(name="psc", bufs=2, space="PSUM"))
    py_p = ctx.enter_context(tc.tile_pool(name="py", bufs=2, space="PSUM"))
    pm_p = ctx.enter_context(tc.tile_pool(name="pm", bufs=2, space="PSUM"))

    # ---- constants / weights (loaded once; host pre-packs to [128, ...] so
    # each is a single contiguous DMA - the serialized ~600ns DMA triggers on
    # the sync queue were delaying batch-0 xT loads by ~20us) ----
    wq_sb = w_p.tile([128, C], F16, tag="wq")
    wk_sb = w_p.tile([128, C], F16, tag="wk")
    wva_sb = w_p.tile([128, NKT * 130], F16, tag="wva")
    nc.sync.dma_start(wq_sb[:], wq[:])
    nc.sync.dma_start(wk_sb[:], wk[:])
    nc.sync.dma_start(wva_sb[:], wva[:])
    bva_bc = const_p.tile([128, 130], F32, tag="bvab")
    m0t = const_p.tile([128, 1], F32, tag="m0")

    xts = [[None] * NKT for _ in range(B)]
    qTs = [None] * B
    kTs = [None] * B
    vas = [[None] * NTT for _ in range(B)]

    qproj = deque()
    qout = deque()
    state = {"step": 0}
    NSTEPS = sum(4 * ic + 4 for ic in range(NIC))  # 40

    def emit_load(b):
        for kt in range(NKT):
            xt = xt_p.tile([128, T], F16, tag="xt")
            nc.sync.dma_start(xt[:], xT[b, kt * 128 : (kt + 1) * 128, :])
            xts[b][kt] = xt

    def make_proj_thunks(b):
        ths = []

        def alloc(b=b):
            qTs[b] = qk_p.tile([128, T], F16, tag="qT", name="qT")
            kTs[b] = qk_p.tile([128, T], F16, tag="kT", name="kT")

        ths.append(alloc)
        for n in range(NIC):
            for which in ("q", "k"):
                def th(b=b, n=n, which=which):
                    csl = slice(n * ICH, (n + 1) * ICH)
                    wsb = wq_sb if which == "q" else wk_sb
                    bias = bias_q if which == "q" else bias_k
                    dst = qTs[b] if which == "q" else kTs[b]
                    ps = pm_p.tile([128, ICH], F32, tag="pm")
                    for kt in range(NKT):
                        nc.tensor.matmul(
                            ps[:],
                            wsb[:, kt * 128 : (kt + 1) * 128],
                            xts[b][kt][:, csl],
                            start=kt == 0,
                            stop=kt == NKT - 1,
                        )
                    nc.vector.tensor_scalar_add(dst[:, csl], ps[:], bias[:])

                ths.append(th)
        for tt in range(NTT):
            def th(b=b, tt=tt):
                tsl = slice(tt * 128, (tt + 1) * 128)
                ps = pm_p.tile([128, 130], F32, tag="pm")
                for kt in range(NKT):
                    nc.tensor.matmul(
                        ps[:],
                        xts[b][kt][:, tsl],
                        wva_sb[:, kt * 130 : (kt + 1) * 130],
                        start=kt == 0,
                        stop=kt == NKT - 1,
                    )
                va = va_p.tile([128, 130], F16, tag="va")
                nc.vector.tensor_add(va[:], ps[:], bva_bc[:])
                vas[b][tt] = va

            ths.append(th)
        return ths

    def make_outproj_thunks(b, ic, yta):
        ths = []
        for itl in range(4):
            def th(b=b, ic=ic, yta=yta, itl=itl):
                it = ic * 4 + itl
                off = itl * 128
                for nch in range(2):
                    osl = slice(nch * ICH, (nch + 1) * ICH)
                    po = pm_p.tile([128, ICH], F32, tag="pm")
                    nc.tensor.matmul(
                        po[:], yta[:, off : off + 128], wo_sb[:, osl],
                        start=True, stop=True,
                    )
                    obt = ob_p.tile([128, ICH], F16, tag="ob")
                    nc.vector.tensor_copy(obt[:], po[:])
                    nc.sync.dma_start(out[b, it * 128 : (it + 1) * 128, osl], obt[:])

            ths.append(th)
        return ths

    def pace(skip_out=False):
        s = state["step"]
        state["step"] = s + 1
        if s >= 8 and qproj:
            if s < NSTEPS - 4:
                k = -(-len(qproj) // (NSTEPS - 4 - s))
            else:
                k = len(qproj)
            for _ in range(min(k, len(qproj))):
                qproj.popleft()()
        # keep the DVE queue shallow around the alpha chain and let the next
        # batch's scores reach the PE FIFO before yta-dependent out-proj MMs
        if qout and not skip_out:
            qout.popleft()()
            if len(qout) > 6:
                qout.popleft()()

    def emit_alpha_half(py):
        """denominator -> alpha for one head; dn staged via ACT (keeps the
        latency-critical chain off the CAST-laden DVE queue; custom-DVE
        reciprocal also misreads PSUM sources so SBUF staging is required)."""
        dn = r_p.tile([1, ICH], F32, tag="dn")
        nc.scalar.activation(dn[:], py[64:65, :], AF.Copy)
        r = r_p.tile([1, ICH], F32, tag="r")
        nc.vector.reciprocal_approx_fast(r[:], dn[:])
        rb = rb_p.tile([64, ICH], F32, tag="rb")
        nc.gpsimd.partition_broadcast(rb[:], r[:])
        return rb

    def do_batch_attention(b):
        steps = []
        for ic in range(NIC):
            njt = 4 * ic + 4
            i0 = ic * ICH
            for jt in range(njt):
                k = jt - 4 * ic
                if k >= 0:
                    wdt, istart = ICH - 128 * k, 128 * jt
                else:
                    wdt, istart = ICH, i0
                steps.append((ic, jt, njt, i0, wdt, istart))

        pscs = {}

        def emit_scores(si):
            ic, jt, njt, i0, wdt, istart = steps[si]
            psc = psc_p.tile([128, 1024], F32, tag="psc")
            jsl = slice(jt * 128, jt * 128 + 128)
            isl = slice(istart, i0 + ICH)
            nc.tensor.matmul(
                psc[:, 0:wdt], kTs[b][0:64, jsl], qTs[b][0:64, isl],
                start=True, stop=True, tile_position=(0, 0),
            )
            nc.tensor.matmul(
                psc[:, 512 : 512 + wdt], kTs[b][64:128, jsl], qTs[b][64:128, isl],
                start=True, stop=True, tile_position=(64, 0),
            )
            pscs[si] = psc

        pys = {}
        emit_scores(0)
        for si in range(len(steps)):
            ic, jt, njt, i0, wdt, istart = steps[si]
            if si + 1 < len(steps):
                emit_scores(si + 1)
            psc = pscs.pop(si)
            e = e_p.tile([128, 1024], F16, tag="e")
            nc.scalar.activation(e[:, 0 : 512 + wdt], psc[:, 0 : 512 + wdt], AF.Exp, bias=m0t[:])
            if jt - 4 * ic >= 0:  # diagonal: zero the j > i triangle (128 cols)
                for off in (0, 512):
                    nc.gpsimd.affine_select(
                        out=e[:, off : off + 128],
                        in_=e[:, off : off + 128],
                        pattern=[[1, 128]],
                        compare_op=mybir.AluOpType.is_ge,
                        fill=0.0,
                        base=0,
                        channel_multiplier=-1,
                    )
            if jt == 0:
                pys[ic] = (
                    py_p.tile([65, ICH], F32, tag="py", name="py0"),
                    py_p.tile([65, ICH], F32, tag="py", name="py1"),
                )
            py0, py1 = pys[ic]
            coff = istart - i0
            st, sp = jt == 0, jt == njt - 1
            nc.tensor.matmul(
                py0[:, coff:ICH], vas[b][jt][:, 0:65], e[:, 0:wdt], start=st, stop=sp
            )
            if sp:
                rb0 = emit_alpha_half(py0)
            nc.tensor.matmul(
                py1[:, coff:ICH], vas[b][jt][:, 65:130], e[:, 512 : 512 + wdt],
                start=st, stop=sp,
            )
            if sp:
                rb1 = emit_alpha_half(py1)
                yta = yta_p.tile([128, ICH], F16, tag="yta")
                nc.vector.tensor_mul(yta[0:64, :], py0[0:64, :], rb0[:])
                nc.vector.tensor_mul(yta[64:128, :], py1[0:64, :], rb1[:])
                pys.pop(ic)
                qout.extend(make_outproj_thunks(b, ic, yta))
            pace(skip_out=sp or si <= 1)

    # ---- pipeline over batches ----
    emit_load(0)
    wo_sb = w_p.tile([128, C], F16, tag="wo")
    nc.sync.dma_start(wo_sb[:], wo[:])
    bias_q = const_p.tile([W, 1], F32, tag="bq")
    bias_k = const_p.tile([W, 1], F32, tag="bk")
    nc.sync.dma_start(bias_q[:], bq[:])
    nc.sync.dma_start(bias_k[:], bk[:])
    bva_row = const_p.tile([1, 130], F32, tag="bvar")
    nc.sync.dma_start(bva_row[:], bva[:])
    nc.gpsimd.partition_broadcast(bva_bc[:], bva_row[:])
    nc.gpsimd.memset(m0t[:], -M0)
    for th in make_proj_thunks(0):
        th()
    for b in range(B):
        if b + 1 < B:
            emit_load(b + 1)
            qproj.extend(make_proj_thunks(b + 1))
        state["step"] = 0
        do_batch_attention(b)
        while qproj:  # proj(b+1) must be complete before attention(b+1)
            qproj.popleft()()
    while qout:
        qout.popleft()()


def _build():
    if "nc" in _CACHE:
        return _CACHE["nc"]
    nc = bacc.Bacc("TRN2", target_bir_lowering=False, debug=False, num_devices=N_CORES)
    xT = nc.dram_tensor("xT", [B, C, T], F16, kind="ExternalInput").ap()
    wq = nc.dram_tensor("wq", [128, C], F16, kind="ExternalInput").ap()
    wk = nc.dram_tensor("wk", [128, C], F16, kind="ExternalInput").ap()
    wva = nc.dram_tensor("wva", [128, NKT * 130], F16, kind="ExternalInput").ap()
    wo = nc.dram_tensor("wo", [W, C], F16, kind="ExternalInput").ap()
    bq = nc.dram_tensor("bq", [W, 1], F32, kind="ExternalInput").ap()
    bk = nc.dram_tensor("bk", [W, 1], F32, kind="ExternalInput").ap()
    bva = nc.dram_tensor("bva", [1, 130], F32, kind="ExternalInput").ap()
    out = nc.dram_tensor("out", [B, T, C], F16, kind="ExternalOutput").ap()

    with tile.TileContext(nc) as tc:
        with ExitStack() as ctx:
            _kernel_body(ctx, tc, xT, wq, wk, wva, wo, bq, bk, bva, out)
    nc.compile()
    _CACHE["nc"] = nc
    return nc


def make_in_maps(inputs):
    x = np.asarray(inputs["x"], np.float32)
    Wq = np.asarray(inputs["Wq"], np.float32)
    bq = np.asarray(inputs["bq"], np.float32)
    Wk = np.asarray(inputs["Wk"], np.float32)
    bk = np.asarray(inputs["bk"], np.float32)
    Wv = np.asarray(inputs["Wv"], np.float32)
    bv = np.asarray(inputs["bv"], np.float32)
    Wo = np.asarray(inputs["Wo"], np.float32)

    scale = np.float32(1.0 / np.sqrt(D))
    xT = np.ascontiguousarray(x.transpose(0, 2, 1)).astype(np.float16)  # [B, C, T]
    Wq_s = Wq * scale
    bq_s = bq * scale

    in_maps = []
    for c in range(N_CORES):
        s = slice(c * W, (c + 1) * W)
        wva = np.zeros((C, 130), np.float32)
        wva[:, 0:64] = Wv[:, c * W : c * W + D]
        wva[:, 65:129] = Wv[:, c * W + D : (c + 1) * W]
        wva_p = wva.reshape(NKT, 128, 130).transpose(1, 0, 2).reshape(128, NKT * 130)
        wq_p = (
            np.ascontiguousarray(Wq_s[:, s])
            .reshape(NKT, 128, W)
            .transpose(1, 0, 2)
            .reshape(128, C)
        )
        wk_p = (
            np.ascontiguousarray(Wk[:, s])
            .reshape(NKT, 128, W)
            .transpose(1, 0, 2)
            .reshape(128, C)
        )
        bva = np.zeros((1, 130), np.float32)
        bva[0, 0:64] = bv[c * W : c * W + D]
        bva[0, 64] = 1.0
        bva[0, 65:129] = bv[c * W + D : (c + 1) * W]
        bva[0, 129] = 1.0
        in_maps.append(
            {
                "xT": xT,
                "wq": np.ascontiguousarray(wq_p).astype(np.float16),
                "wk": np.ascontiguousarray(wk_p).astype(np.float16),
                "wva": np.ascontiguousarray(wva_p).astype(np.float16),
                "wo": np.ascontiguousarray(Wo[s, :]).astype(np.float16),
                "bq": np.ascontiguousarray(bq_s[s, None]),
                "bk": np.ascontiguousarray(bk[s, None]),
                "bva": bva,
            }
        )
    return in_maps


def kernel(**inputs):
    nc = _build()
    in_maps = make_in_maps(inputs)
    res = bass_utils.run_bass_kernel_spmd(nc, in_maps, core_ids=list(range(N_CORES)))
    bo = np.asarray(inputs["bo"], np.float32)
    out = np.zeros((B, T, C), np.float32)
    for c in range(N_CORES):
        out += res.results[c]["out"].astype(np.float32)
    out += bo
    return out


if __name__ == "__main__":
    rng = np.random.default_rng(0)
    ins = {
        "x": rng.standard_normal((B, T, C), dtype=np.float32),
        "Wq": rng.standard_normal((C, C), dtype=np.float32) / 32,
        "bq": rng.standard_normal((C,), dtype=np.float32) * 0.02,
        "Wk": rng.standard_normal((C, C), dtype=np.float32) / 32,
        "bk": rng.standard_normal((C,), dtype=np.float32) * 0.02,
        "Wv": rng.standard_normal((C, C), dtype=np.float32) / 32,
        "bv": rng.standard_normal((C,), dtype=np.float32) * 0.02,
        "Wo": rng.standard_normal((C, C), dtype=np.float32) / 32,
        "bo": rng.standard_normal((C,), dtype=np.float32) * 0.02,
    }
    got = kernel(**ins)
    print("kernel ran, out shape", got.shape)
